# revision 1
# baseline (speedup 1.0000x reference)
"""ExternalAttention kernel for Trainium2 (8 NeuronCores, batch-parallel).

Math (collapsed from the reference nn.Module):
  q = (poi_data @ wq1 + bq1)[:, 0] @ wq2 + bq2            # [512], shared
  per head h: wkq[:, h] = wk[:, 64h:64h+64] @ q[64h:64h+64] # [512, 8]
  scores = x @ wkq  (+ const per head -- cancels in softmax)
  A = softmax(scores / 8, axis=L)
  xa[h, :] = sum_l A[l, h] * x[l, :]                       # [8, 512]
  V[64h:64h+64] = xa[h] @ wv[:, 64h:64h+64]                # [512]
  row = (V / Z) @ wo + (bv @ wo + bo)                      # [512]
  out[b, l, :] = row_b  for every l.

Sharding: data-parallel over B (8 batch elements = 8 cores); the tiny
shared weights are replicated. Each core streams its x_b once from HBM
through a software-pipelined transpose/score/accumulate loop, then
projects and broadcast-writes the single output row.
"""

import os
import sys

import numpy as np

for _p in ("/opt/trn_rl_repo", "/opt/pypackages"):
    if os.path.isdir(_p) and _p not in sys.path:
        sys.path.append(_p)

B, L, D = 8, 8192, 512
H, DH = 8, 64
P = 128
NCHUNK = L // P  # 64
NJ = D // P  # 4
SCALE = 1.0 / np.sqrt(DH)  # 0.125
N_CORES = 8

_CACHE = {}


def _build_bass():
    import concourse.bass as bass
    import concourse.tile as tile
    from concourse import mybir
    from concourse.bacc import Bacc

    f32 = mybir.dt.float32
    ts = bass.ts

    nc = Bacc(num_swdge_queues=4)
    x_d = nc.dram_tensor("x", [L, D], f32, kind="ExternalInput")
    wkq_d = nc.dram_tensor("wkq", [D, H], f32, kind="ExternalInput")
    wv_d = nc.dram_tensor("wv", [D, D], f32, kind="ExternalInput")
    wo_d = nc.dram_tensor("wo", [D, D], f32, kind="ExternalInput")
    bo2_d = nc.dram_tensor("bo2", [1, D], f32, kind="ExternalInput")
    id_d = nc.dram_tensor("ident", [P, P], f32, kind="ExternalInput")
    m84_d = nc.dram_tensor("m84", [H, NJ], f32, kind="ExternalInput")
    s82_d = nc.dram_tensor("s82", [H, 2], f32, kind="ExternalInput")
    ea2_d = nc.dram_tensor("ea2", [2, P], f32, kind="ExternalInput")
    row_d = nc.dram_tensor("row_scratch", [1, D], f32)
    out_d = nc.dram_tensor("out", [L, D], f32, kind="ExternalOutput")

    with tile.TileContext(nc) as tc:
        with (
            tc.tile_pool(name="consts", bufs=1) as consts,
            tc.tile_pool(name="xin", bufs=16) as xin,
            tc.tile_pool(name="xt", bufs=10) as xtp,
            tc.tile_pool(name="pp", bufs=10) as ppp,
            tc.tile_pool(name="epi", bufs=1) as epi,
        ):
            id128 = consts.tile([P, P], f32)
            nc.scalar.dma_start(id128, id_d[:])
            id1 = consts.tile([1, 1], f32)
            nc.vector.memset(id1, 1.0)
            ones_col = consts.tile([P, 1], f32)
            nc.vector.memset(ones_col, 1.0)

            wkq_sb = consts.tile([P, NJ, H], f32)
            nc.scalar.dma_start(wkq_sb, wkq_d.rearrange("(j p) h -> p j h", p=P))
            wv_sb = consts.tile([P, NJ, D], f32)
            wo_sb = consts.tile([P, NJ, D], f32)
            bo2_sb = consts.tile([1, D], f32)
            m84_sb = consts.tile([H, NJ], f32)
            s82_sb = consts.tile([H, 2], f32)
            ea2_sb = consts.tile([2, P], f32)

            # Per-partition partial softmax denominators, summed over
            # partitions once in the epilogue.
            zacc_sb = epi.tile([P, H], f32)
            nc.vector.memset(zacc_sb, 0.0)

            xa_sb = epi.tile([P, NJ, H], f32)
            z128_sb = epi.tile([P, NJ], f32)

            with tc.tile_pool(name="ps_acc", bufs=1, space="PSUM") as ps_acc:
                # Persistent xa^T accumulators, one PSUM bank per d-slice
                # so each holds exactly one open accumulation group.
                xa_ps = [
                    ps_acc.tile([P, H], f32, name=f"xa{j}", tag=f"xa{j}")
                    for j in range(NJ)
                ]

                with (
                    tc.tile_pool(name="ps_t", bufs=3, space="PSUM") as ps_t,
                    tc.tile_pool(name="ps_s", bufs=1, space="PSUM") as ps_s,
                ):
                    xv = x_d.rearrange("(n p) d -> n p d", p=P)
                    # Software pipeline with a 2-step skew so PE never waits
                    # on the DVE/ACT copy or the exp between its own
                    # instructions: step c = transpose(c), scores(c-1),
                    # accumulate(c-2).
                    xs, xts, ps = {}, {}, {}
                    for c in range(NCHUNK + 2):
                        if c < NCHUNK:
                            x_t = xin.tile([P, D], f32)
                            if c == 0:
                                # split the first load so the pipeline fills
                                # as fast as both queues allow
                                nc.sync.dma_start(x_t[:, 0:256], xv[c][:, 0:256])
                                nc.gpsimd.dma_start(
                                    x_t[:, 256:D], xv[c][:, 256:D]
                                )
                            else:
                                dma_eng = nc.sync if c % 2 == 0 else nc.gpsimd
                                dma_eng.dma_start(x_t, xv[c])
                            xs[c] = x_t

                            xt_ps = ps_t.tile([P, D], f32)
                            for j in range(NJ):
                                nc.tensor.transpose(
                                    xt_ps[:, ts(j, P)], x_t[:, ts(j, P)], id128
                                )
                            xt_sb = xtp.tile([P, D], f32)
                            nc.vector.tensor_copy(
                                xt_sb[:, 0:344], xt_ps[:, 0:344]
                            )
                            nc.scalar.copy(xt_sb[:, 344:D], xt_ps[:, 344:D])
                            xts[c] = xt_sb

                        if 1 <= c <= NCHUNK:
                            cc = c - 1
                            s_ps = ps_s.tile([P, H], f32)
                            for j in range(NJ):
                                nc.tensor.matmul(
                                    s_ps,
                                    xts[cc][:, ts(j, P)],
                                    wkq_sb[:, j, :],
                                    start=(j == 0),
                                    stop=(j == NJ - 1),
                                )
                            p_sb = ppp.tile([P, H], f32)
                            nc.scalar.activation(
                                p_sb,
                                s_ps,
                                mybir.ActivationFunctionType.Exp,
                                scale=SCALE,
                            )
                            ps[cc] = p_sb

                        if c >= 2:
                            cc = c - 2
                            nc.gpsimd.tensor_add(zacc_sb, zacc_sb, ps[cc])
                            for j in range(NJ):
                                nc.tensor.matmul(
                                    xa_ps[j],
                                    xs[cc][:, ts(j, P)],
                                    ps[cc],
                                    start=(cc == 0),
                                    stop=(cc == NCHUNK - 1),
                                )
                            del xs[cc], ps[cc]
                            if cc - 1 in xts:
                                del xts[cc - 1]

                # epilogue-only constants -- load after the stream
                nc.sync.dma_start(wv_sb, wv_d.rearrange("(j p) n -> p j n", p=P))
                nc.sync.dma_start(wo_sb, wo_d.rearrange("(j p) n -> p j n", p=P))
                nc.gpsimd.dma_start(bo2_sb, bo2_d[:])
                nc.gpsimd.dma_start(m84_sb, m84_d[:])
                nc.gpsimd.dma_start(s82_sb, s82_d[:])
                nc.gpsimd.dma_start(ea2_sb, ea2_d[:])

                # drain accumulators; build the [128, 4] normalization grid
                # z128[p, j] = 1 / Z[2j + p//64] from Z via two 0/1 matmuls
                with tc.tile_pool(name="pe0", bufs=1, space="PSUM") as pe0:
                    for j in range(NJ):
                        nc.vector.tensor_copy(xa_sb[:, j, :], xa_ps[j])

                    z_ps = pe0.tile([1, H], f32, tag="t0")
                    nc.tensor.matmul(z_ps, ones_col, zacc_sb)
                    zr_sb = epi.tile([1, H], f32)
                    nc.vector.reciprocal(zr_sb, z_ps)

                    zrt_ps = pe0.tile([H, 1], f32, tag="t0")
                    nc.tensor.transpose(zrt_ps, zr_sb, id1)
                    zrt_sb = epi.tile([H, 1], f32)
                    nc.vector.tensor_copy(zrt_sb, zrt_ps)

                    b_sb = epi.tile([H, NJ], f32)
                    nc.vector.tensor_scalar_mul(b_sb, m84_sb, zrt_sb)
                    r2_ps = pe0.tile([2, NJ], f32, tag="t0")
                    nc.tensor.matmul(r2_ps, s82_sb, b_sb)
                    r2_sb = epi.tile([2, NJ], f32)
                    nc.vector.tensor_copy(r2_sb, r2_ps)
                    z128_ps = pe0.tile([P, NJ], f32, tag="t0")
                    nc.tensor.matmul(z128_ps, ea2_sb, r2_sb)
                    nc.vector.tensor_copy(z128_sb, z128_ps)

            # ---- project V directly in transposed [128, .] layout ----
            with tc.tile_pool(name="pe1", bufs=1, space="PSUM") as pe1:
                vt_sb = epi.tile([P, NJ], f32)
                for j in range(NJ):
                    vtj = pe1.tile([P, 2], f32, name=f"vt{j}", tag=f"vt{j}")
                    for k in range(NJ):
                        nc.tensor.matmul(
                            vtj,
                            wv_sb[:, k, ts(j, P)],
                            xa_sb[:, k, 2 * j : 2 * j + 2],
                            start=(k == 0),
                            stop=(k == NJ - 1),
                        )
                    # h = 2j + p//64: lower half takes column 0, upper column 1
                    nc.vector.tensor_copy(vt_sb[0:64, j : j + 1], vtj[0:64, 0:1])
                    nc.vector.tensor_copy(
                        vt_sb[64:P, j : j + 1], vtj[64:P, 1:2]
                    )

                vtn_sb = epi.tile([P, NJ], f32)
                nc.vector.tensor_mul(vtn_sb, vt_sb, z128_sb)

                row_ps = pe1.tile([1, D], f32, tag="row")
                for j in range(NJ):
                    nc.tensor.matmul(
                        row_ps,
                        vtn_sb[:, j : j + 1],
                        wo_sb[:, j, :],
                        start=(j == 0),
                        stop=(j == NJ - 1),
                    )
                row_sb = epi.tile([1, D], f32)
                nc.vector.tensor_add(row_sb, row_ps, bo2_sb)

                # broadcast write: bounce the row through DRAM, fill a
                # [128, 4, 512] SBUF tile (4 row copies per partition) via a
                # DRAM-side stride-0 broadcast, then write the output as 16
                # one-MB DMAs whose per-partition runs are 8 KB contiguous.
                r_sb = epi.tile([P, D], f32)
                nc.gpsimd.partition_broadcast(r_sb, row_sb)
                ov = out_d.rearrange("(n p) d -> n p d", p=P)
                w_engines = [nc.sync, nc.gpsimd, nc.scalar]
                for c in range(NCHUNK):
                    w_engines[c % len(w_engines)].dma_start(ov[c], r_sb)

    if not nc.is_finalized():
        nc.finalize()
    return nc


def _get_nc():
    if "nc" not in _CACHE:
        _CACHE["nc"] = _build_bass()
    return _CACHE["nc"]


def _host_prep(inputs):
    poi = np.asarray(inputs["poi_data"], np.float32)
    wq1 = np.asarray(inputs["wq1"], np.float32)
    bq1 = np.asarray(inputs["bq1"], np.float32)
    wq2 = np.asarray(inputs["wq2"], np.float32)
    bq2 = np.asarray(inputs["bq2"], np.float32)
    wk = np.asarray(inputs["wk"], np.float32)

    q1 = (poi @ wq1 + bq1)[:, 0]  # [1683]
    q = q1 @ wq2 + bq2  # [512]
    qh = q.reshape(H, DH)
    wkq = np.stack(
        [wk[:, h * DH : (h + 1) * DH] @ qh[h] for h in range(H)], axis=1
    )  # [512, 8]
    return wkq.astype(np.float32)


def _make_in_maps(inputs):
    x = np.ascontiguousarray(np.asarray(inputs["x"], np.float32))
    wv = np.ascontiguousarray(np.asarray(inputs["wv"], np.float32))
    wo = np.ascontiguousarray(np.asarray(inputs["wo"], np.float32))
    bv = np.asarray(inputs["bv"], np.float32).reshape(D)
    bo = np.asarray(inputs["bo"], np.float32).reshape(D)
    wkq = _host_prep(inputs)

    bo2 = (bv @ wo + bo).reshape(1, D).astype(np.float32)
    hh = np.arange(H)
    m84 = (hh[:, None] // 2 == np.arange(NJ)[None, :]).astype(np.float32)
    s82 = (hh[:, None] % 2 == np.arange(2)[None, :]).astype(np.float32)
    ea2 = (np.arange(2)[:, None] == (np.arange(P)[None, :] // 64)).astype(
        np.float32
    )
    ident = np.eye(P, dtype=np.float32)

    return [
        {
            "x": np.ascontiguousarray(x[b]),
            "wkq": wkq,
            "wv": wv,
            "wo": wo,
            "bo2": bo2,
            "ident": ident,
            "m84": m84,
            "s82": s82,
            "ea2": ea2,
        }
        for b in range(N_CORES)
    ]


def kernel(**inputs) -> np.ndarray:
    from concourse.bass_utils import run_bass_kernel_spmd

    nc = _get_nc()
    in_maps = _make_in_maps(inputs)
    res = run_bass_kernel_spmd(nc, in_maps, list(range(N_CORES)))
    out = np.stack([res.results[b]["out"] for b in range(N_CORES)], axis=0)
    return out.astype(np.float32)



# revision 3
# speedup vs baseline: 1.0108x; 1.0108x over previous
"""ExternalAttention kernel for Trainium2 (8 NeuronCores, batch-parallel).

Math (collapsed from the reference nn.Module):
  q = (poi_data @ wq1 + bq1)[:, 0] @ wq2 + bq2            # [512], shared
  per head h: wkq[:, h] = wk[:, 64h:64h+64] @ q[64h:64h+64] # [512, 8]
  scores = x @ wkq  (+ const per head -- cancels in softmax)
  p = exp(scores / 8);  Z[h] = sum_l p[l, h]
  xa[d, h] = sum_l p[l, h] * x[l, d]
  V[64h+e] = sum_d (xa[d, h]/Z[h]) * wv[d, 64h+e]
  row = V @ wo + (bv @ wo + bo)
  out[b, l, :] = row_b  for every l.

Data-parallel over B (one batch element per core).  Per core the kernel
streams x twice from HBM in fp8-e4m3 -- once row-major (for the xa
accumulation, contraction over L on partitions) and once transposed
host-side (for the scores matmul, contraction over D on partitions) --
so no on-chip transpose is needed.  Weights travel in fp16.  The scores
stream is loaded first so the softmax work finishes mid-stream; only
the final xa accumulation trails the last x load.  The final row is
accumulated onto a host-preloaded bias row in DRAM and broadcast to all
8192 output rows by a stride-0-source DRAM->DRAM DMA.
"""

import os
import sys

import numpy as np

for _p in ("/opt/trn_rl_repo", "/opt/pypackages"):
    if os.path.isdir(_p) and _p not in sys.path:
        sys.path.append(_p)

B, L, D = 8, 8192, 512
H, DH = 8, 64
P = 128
NJ = D // P  # 4 d-blocks
NCHUNK = L // P  # 64 chunks of 128 rows
GL = 4  # chunks per load DMA
NG = NCHUNK // GL  # 16 load groups per stream
GE = 32  # chunks per exp group
NE = NCHUNK // GE  # 2 exp groups
SCALE = 1.0 / np.sqrt(DH)  # 0.125
N_CORES = 8

# schedule constants (tuned against the CoreSim cost model)
ACT_PRECHARGE = 2080.0  # scalar-queue precharge for its exp work
EXP_AFTER = (4, 6)  # exp e emitted after scalar's k-th x8-phase DMA
W_POS = "late"  # weights load after the x8 stream
W_FP8 = False
W_HALVES = True

_CACHE = {}


def _build_bass():
    import concourse.bass as bass
    import concourse.tile as tile
    from concourse import mybir
    from concourse.bacc import Bacc

    f32 = mybir.dt.float32
    f16 = mybir.dt.float16
    f8 = mybir.dt.float8e4
    ts = bass.ts

    nc = Bacc(num_swdge_queues=4)
    x8_d = nc.dram_tensor("x8", [L, D], f8, kind="ExternalInput")
    xt8_d = nc.dram_tensor("xt8", [L, D], f8, kind="ExternalInput")
    wkq_d = nc.dram_tensor("wkq", [D, H + 1], f16, kind="ExternalInput")
    wdt = f8 if W_FP8 else f16
    wv_d = nc.dram_tensor("wv", [D, D], wdt, kind="ExternalInput")
    wo_d = nc.dram_tensor("wo", [D, D], wdt, kind="ExternalInput")
    row_d = nc.dram_tensor("row_scratch", [1, D], f32)
    out_d = nc.dram_tensor("out", [L, D], f32, kind="ExternalOutput")

    with tile.TileContext(nc) as tc:
        with (
            tc.tile_pool(name="consts", bufs=1) as consts,
            tc.tile_pool(name="xin", bufs=1) as xin,
            tc.tile_pool(name="xtin", bufs=1) as xtin,
            tc.tile_pool(name="pp", bufs=2) as ppp,
            tc.tile_pool(name="epi", bufs=1) as epi,
        ):
            wkq_sb = consts.tile([P, NJ, H + 1], f16)
            nc.sync.dma_start(wkq_sb, wkq_d.rearrange("(j p) h -> p j h", p=P))
            ones_mat = consts.tile([P, P], f16)
            nc.vector.memset(ones_mat, 1.0)

            wv_sb = consts.tile([P, NJ, D], wdt)
            wo_sb = consts.tile([P, NJ, D], wdt)

            # DMA queue schedule, greedy-balanced over the three queues.
            # The transposed (scores) stream loads first; then the two
            # projection weights; then the row-major stream, so only the
            # final xa accumulation trails the stream's tail.
            x8v = x8_d.rearrange("(g c p) e -> g p c e", p=P, c=GL)
            xtv = xt8_d.rearrange("(g c p) e -> g p c e", p=P, c=GL)
            wvv = wv_d.rearrange("(j p) n -> p j n", p=P)
            wov = wo_d.rearrange("(j p) n -> p j n", p=P)
            engs = [nc.sync, nc.scalar, nc.gpsimd]
            load = [500.0, ACT_PRECHARGE, 0.0]
            act_dma_count = [0]

            def pick():
                qi = load.index(min(load))
                if qi == 1:
                    act_dma_count[0] += 1
                return qi

            xtt = []
            for g in range(NG):
                t2 = xtin.tile([P, GL, D], f8, name=f"xtg{g}")
                qi = pick()
                engs[qi].dma_start(t2, xtv[g])
                load[qi] += 790.0
                xtt.append(t2)

            def emit_weights():
                if W_HALVES:
                    for sb, dv in ((wv_sb, wvv), (wo_sb, wov)):
                        for hj in range(2):
                            qi = pick()
                            engs[qi].dma_start(
                                sb[:, 2 * hj : 2 * hj + 2, :],
                                dv[:, 2 * hj : 2 * hj + 2, :],
                            )
                            load[qi] += 790.0
                else:
                    wcost = 790.0 if W_FP8 else 1579.0
                    qi = pick()
                    engs[qi].dma_start(wv_sb, wvv)
                    load[qi] += wcost
                    qi = pick()
                    engs[qi].dma_start(wo_sb, wov)
                    load[qi] += wcost

            if W_POS == "early":
                emit_weights()

            x8t = [None] * NG

            with (
                tc.tile_pool(name="ps_acc", bufs=1, space="PSUM") as ps_acc,
                tc.tile_pool(name="ps_s", bufs=2, space="PSUM") as ps_s,
            ):
                xa_ps = [
                    ps_acc.tile([P, H], f32, name=f"xa{j}", tag=f"xa{j}")
                    for j in range(NJ)
                ]
                z_ps = ps_acc.tile([P, H], f32, name="z", tag="z")

                # scores for both exp groups (PE-only; paced by xt arrivals)
                s_ts, p_ts = [], []
                for e in range(NE):
                    s_t = ps_s.tile([P, GE * H], f32)
                    for ci in range(GE):
                        c = e * GE + ci
                        lt = xtt[c // GL]
                        for j in range(NJ):
                            nc.tensor.matmul(
                                s_t[:, ci * H : ci * H + H],
                                lt[:, c % GL, ts(j, P)],
                                wkq_sb[:, j, 0:H],
                                start=(j == 0),
                                stop=(j == NJ - 1),
                            )
                    s_ts.append(s_t)

                def emit_exp(e):
                    p_t = ppp.tile([P, GE * H], f16)
                    nc.scalar.activation(
                        p_t,
                        s_ts[e],
                        mybir.ActivationFunctionType.Exp,
                        scale=SCALE,
                    )
                    p_ts.append(p_t)

                # x8 loads; exps slot into the scalar (ACT) queue after its
                # EXP_AFTER[k]-th x8-phase DMA so the softmax weights are
                # ready while the x8 stream is still in flight.
                exps_done = 0
                act_dma_count[0] = 0
                for g in range(NG):
                    if W_POS == "mid" and g == NG - 2:
                        emit_weights()
                    t = xin.tile([P, GL, D], f8, name=f"x8g{g}")
                    qi = pick()
                    engs[qi].dma_start(t, x8v[g])
                    load[qi] += 790.0
                    x8t[g] = t
                    while exps_done < NE and exps_done < len(
                        [k for k in EXP_AFTER if k <= act_dma_count[0]]
                    ):
                        emit_exp(exps_done)
                        exps_done += 1
                for e in range(exps_done, NE):
                    emit_exp(e)
                if W_POS == "late":
                    emit_weights()

                # Z (replicated over partitions) then xa accumulation, in
                # chunk order so the PE paces with the x8 stream.
                for c in range(NCHUNK):
                    nc.tensor.matmul(
                        z_ps,
                        ones_mat,
                        p_ts[c // GE][:, (c % GE) * H : (c % GE) * H + H],
                        start=(c == 0),
                        stop=(c == NCHUNK - 1),
                    )
                for c in range(NCHUNK):
                    xt_ = x8t[c // GL]
                    pr = p_ts[c // GE][:, (c % GE) * H : (c % GE) * H + H]
                    for j in range(NJ):
                        nc.tensor.matmul(
                            xa_ps[j],
                            xt_[:, c % GL, ts(j, P)],
                            pr,
                            start=(c == 0),
                            stop=(c == NCHUNK - 1),
                        )

                # ---- epilogue ----
                # xa drains split DVE/ACT; 1/Z on DVE afterwards (hides
                # under the PE's vtall matmuls).
                zrec_sb = epi.tile([P, H], f32)
                nc.vector.reciprocal(zrec_sb, z_ps)
                xa_sb = epi.tile([P, NJ, H], f16)
                nc.vector.tensor_copy(xa_sb[:, 0, :], xa_ps[0])
                nc.scalar.copy(xa_sb[:, 1, :], xa_ps[1])
                nc.vector.tensor_copy(xa_sb[:, 2, :], xa_ps[2])
                nc.scalar.copy(xa_sb[:, 3, :], xa_ps[3])

            with tc.tile_pool(name="pe1", bufs=1, space="PSUM") as pe1:
                # V^T-ish accumulator: vtall[p, 2j+c] = sum_d wv[d, 128j+p']
                # xa_n[d, 2j+c]; only the (c == p//64) columns matter.
                vtall_ps = pe1.tile([P, 2 * NJ], f32, tag="vtall")
                for j in range(NJ):
                    for k in range(NJ):
                        nc.tensor.matmul(
                            vtall_ps[:, 2 * j : 2 * j + 2],
                            wv_sb[:, k, ts(j, P)],
                            xa_sb[:, k, 2 * j : 2 * j + 2],
                            start=(k == 0),
                            stop=(k == NJ - 1),
                        )
                vtn_sb = epi.tile([P, 2 * NJ], f16)
                nc.vector.tensor_mul(vtn_sb, vtall_ps, zrec_sb)

                # row^T [128, 4]: rowT[p, nb] = row[128 nb + p]; the two
                # partition halves select even/odd head columns of vtn.
                rowT_ps = pe1.tile([P, NJ], f32, tag="rowt")
                for nb in range(NJ):
                    mm = 0
                    for j in range(NJ):
                        for hb in range(2):
                            sl = slice(64 * hb, 64 * hb + 64)
                            nc.tensor.matmul(
                                rowT_ps[:, nb : nb + 1],
                                wo_sb[sl, j, ts(nb, P)],
                                vtn_sb[sl, 2 * j + hb : 2 * j + hb + 1],
                                start=(mm == 0),
                                stop=(mm == 7),
                            )
                            mm += 1
                rowT_sb = epi.tile([P, NJ], f32)
                nc.vector.tensor_add(rowT_sb, rowT_ps, wkq_sb[:, :, H])

                # bounce row^T -> DRAM row, then one stride-0 broadcast DMA
                # (same queue, so its DGE delay overlaps the bounce) writes
                # all 8192 identical output rows.
                rview = bass.AP(row_d[:].tensor, 0, [[1, P], [P, NJ]])
                nc.sync.dma_start(rview, rowT_sb[:])
                nc.sync.dma_start(out_d[:], row_d[:].broadcast_to((L, D)))

    if not nc.is_finalized():
        nc.finalize()
    return nc


def _get_nc():
    if "nc" not in _CACHE:
        _CACHE["nc"] = _build_bass()
    return _CACHE["nc"]


def _host_prep(inputs):
    poi = np.asarray(inputs["poi_data"], np.float32)
    wq1 = np.asarray(inputs["wq1"], np.float32)
    bq1 = np.asarray(inputs["bq1"], np.float32)
    wq2 = np.asarray(inputs["wq2"], np.float32)
    bq2 = np.asarray(inputs["bq2"], np.float32)
    wk = np.asarray(inputs["wk"], np.float32)

    q1 = (poi @ wq1 + bq1)[:, 0]  # [1683]
    q = q1 @ wq2 + bq2  # [512]
    qh = q.reshape(H, DH)
    wkq = np.stack(
        [wk[:, h * DH : (h + 1) * DH] @ qh[h] for h in range(H)], axis=1
    )  # [512, 8]
    return wkq


def _make_in_maps(inputs):
    import ml_dtypes

    f8 = ml_dtypes.float8_e4m3
    wdt = f8 if W_FP8 else np.float16
    x = np.asarray(inputs["x"], np.float32)
    wv = np.asarray(inputs["wv"], np.float32).astype(wdt)
    wo32 = np.asarray(inputs["wo"], np.float32)
    wo = wo32.astype(wdt)
    bv = np.asarray(inputs["bv"], np.float32).reshape(D)
    bo = np.asarray(inputs["bo"], np.float32).reshape(D)
    bo2 = (bv @ wo32 + bo).reshape(D).astype(np.float32)
    # wkq plus a 9th column: bo2t[p, j] = bo2[128 j + p], head-block packed
    wkq9 = np.zeros((D, H + 1), np.float16)
    wkq9[:, 0:H] = _host_prep(inputs).astype(np.float16)
    # column H holds bo2 so that wkq9.rearrange((j p) h)[p, j, H] = bo2[128j+p]
    wkq9[:, H] = bo2.reshape(NJ, P).T.reshape(-1, order="F")

    x8 = x.astype(f8)  # [B, L, D]
    # transposed stream: row (n*128 + p) of xt8[b] holds, for chunk n and
    # d-block layout (j, q): xt[n, p, j*128 + q] = x[b, n*128 + q, j*128 + p]
    xt = np.ascontiguousarray(
        x8.reshape(B, NCHUNK, P, NJ, P).transpose(0, 1, 4, 3, 2)
    ).reshape(B, L, D)

    return [
        {
            "x8": np.ascontiguousarray(x8[b]),
            "xt8": xt[b],
            "wkq": wkq9,
            "wv": wv,
            "wo": wo,
        }
        for b in range(N_CORES)
    ]


def kernel(**inputs) -> np.ndarray:
    from concourse.bass_utils import run_bass_kernel_spmd

    nc = _get_nc()
    in_maps = _make_in_maps(inputs)
    res = run_bass_kernel_spmd(nc, in_maps, list(range(N_CORES)))
    out = np.stack([res.results[b]["out"] for b in range(N_CORES)], axis=0)
    return out.astype(np.float32)


# revision 4
# speedup vs baseline: 1.0927x; 1.0810x over previous
"""ExternalAttention kernel for Trainium2 (8 NeuronCores, batch-parallel).

Math (collapsed from the reference nn.Module):
  q = (poi_data @ wq1 + bq1)[:, 0] @ wq2 + bq2            # [512], shared
  per head h: wkq[:, h] = wk[:, 64h:64h+64] @ q[64h:64h+64] # [512, 8]
  scores = x @ wkq  (+ const per head -- cancels in softmax)
  p = exp(scores / 8);  Z[h] = sum_l p[l, h]
  xa[d, h] = sum_l p[l, h] * x[l, d]
  V[64h+e] = sum_d (xa[d, h]/Z[h]) * wv[d, 64h+e]
  row = V @ wo + (bv @ wo + bo)
  out[b, l, :] = row_b  for every l.

Data-parallel over B (one batch element per core).  Per core the kernel
streams x twice from HBM in fp8-e4m3 -- once row-major (for the xa
accumulation, contraction over L on partitions) and once transposed
host-side (for the scores matmul, contraction over D on partitions) --
so no on-chip transpose is needed.  Weights travel in fp16.  The scores
stream is loaded first so the softmax work finishes mid-stream; only
the final xa accumulation trails the last x load.  The final row is
accumulated onto a host-preloaded bias row in DRAM and broadcast to all
8192 output rows by a stride-0-source DRAM->DRAM DMA.
"""

import os
import sys

import numpy as np

for _p in ("/opt/trn_rl_repo", "/opt/pypackages"):
    if os.path.isdir(_p) and _p not in sys.path:
        sys.path.append(_p)

B, L, D = 8, 8192, 512
H, DH = 8, 64
P = 128
NJ = D // P  # 4 d-blocks
NCHUNK = L // P  # 64 chunks of 128 rows
GL = 4  # chunks per load DMA
NG = NCHUNK // GL  # 16 load groups per stream
GE = 32  # chunks per exp group
NE = NCHUNK // GE  # 2 exp groups
SCALE = 1.0 / np.sqrt(DH)  # 0.125
N_CORES = 8

# schedule constants (tuned against the CoreSim cost model)
ACT_PRECHARGE = 2080.0  # scalar-queue precharge for its exp work
EXP_AFTER = (4, 6)  # exp e emitted after scalar's k-th x8-phase DMA
W_POS = "late"  # weights load after the x8 stream
W_FP8 = False
W_HALVES = True

_CACHE = {}


def _build_bass():
    import concourse.bass as bass
    import concourse.tile as tile
    from concourse import mybir
    from concourse.bacc import Bacc

    f32 = mybir.dt.float32
    f16 = mybir.dt.float16
    f8 = mybir.dt.float8e4
    ts = bass.ts

    nc = Bacc(num_swdge_queues=4)
    x8_d = nc.dram_tensor("x8", [L, D], f8, kind="ExternalInput")
    xt8_d = nc.dram_tensor("xt8", [L, D], f8, kind="ExternalInput")
    wkq_d = nc.dram_tensor("wkq", [D, H + 1], f16, kind="ExternalInput")
    wdt = f8 if W_FP8 else f16
    wv_d = nc.dram_tensor("wv", [D, D], wdt, kind="ExternalInput")
    wo_d = nc.dram_tensor("wo", [D, D], wdt, kind="ExternalInput")
    row_d = nc.dram_tensor("row_scratch", [1, D], f16)
    out_d = nc.dram_tensor("out", [L, D], f16, kind="ExternalOutput")

    with tile.TileContext(nc) as tc:
        with (
            tc.tile_pool(name="consts", bufs=1) as consts,
            tc.tile_pool(name="xin", bufs=1) as xin,
            tc.tile_pool(name="xtin", bufs=1) as xtin,
            tc.tile_pool(name="pp", bufs=2) as ppp,
            tc.tile_pool(name="epi", bufs=1) as epi,
        ):
            wkq_sb = consts.tile([P, NJ, H + 1], f16)
            nc.sync.dma_start(wkq_sb, wkq_d.rearrange("(j p) h -> p j h", p=P))
            ones_mat = consts.tile([P, P], f16)
            nc.vector.memset(ones_mat, 1.0)

            wv_sb = consts.tile([P, NJ, D], wdt)
            wo_sb = consts.tile([P, NJ, D], wdt)

            # DMA queue schedule, greedy-balanced over the three queues.
            # The transposed (scores) stream loads first; then the two
            # projection weights; then the row-major stream, so only the
            # final xa accumulation trails the stream's tail.
            x8v = x8_d.rearrange("(g c p) e -> g p c e", p=P, c=GL)
            xtv = xt8_d.rearrange("(g c p) e -> g p c e", p=P, c=GL)
            wvv = wv_d.rearrange("(j p) n -> p j n", p=P)
            wov = wo_d.rearrange("(j p) n -> p j n", p=P)
            engs = [nc.sync, nc.scalar, nc.gpsimd]
            load = [500.0, ACT_PRECHARGE, 0.0]
            act_dma_count = [0]

            def pick():
                qi = load.index(min(load))
                if qi == 1:
                    act_dma_count[0] += 1
                return qi

            xtt = []
            for g in range(NG):
                t2 = xtin.tile([P, GL, D], f8, name=f"xtg{g}")
                qi = pick()
                engs[qi].dma_start(t2, xtv[g])
                load[qi] += 790.0
                xtt.append(t2)

            def emit_weights():
                if W_HALVES:
                    for sb, dv in ((wv_sb, wvv), (wo_sb, wov)):
                        for hj in range(2):
                            qi = pick()
                            engs[qi].dma_start(
                                sb[:, 2 * hj : 2 * hj + 2, :],
                                dv[:, 2 * hj : 2 * hj + 2, :],
                            )
                            load[qi] += 790.0
                else:
                    wcost = 790.0 if W_FP8 else 1579.0
                    qi = pick()
                    engs[qi].dma_start(wv_sb, wvv)
                    load[qi] += wcost
                    qi = pick()
                    engs[qi].dma_start(wo_sb, wov)
                    load[qi] += wcost

            if W_POS == "early":
                emit_weights()

            x8t = [None] * NG

            with (
                tc.tile_pool(name="ps_acc", bufs=1, space="PSUM") as ps_acc,
                tc.tile_pool(name="ps_s", bufs=2, space="PSUM") as ps_s,
            ):
                xa_ps = [
                    ps_acc.tile([P, H], f32, name=f"xa{j}", tag=f"xa{j}")
                    for j in range(NJ)
                ]
                z_ps = ps_acc.tile([P, H], f32, name="z", tag="z")

                # scores for both exp groups (PE-only; paced by xt arrivals)
                s_ts, p_ts = [], []
                for e in range(NE):
                    s_t = ps_s.tile([P, GE * H], f32)
                    for ci in range(GE):
                        c = e * GE + ci
                        lt = xtt[c // GL]
                        for j in range(NJ):
                            nc.tensor.matmul(
                                s_t[:, ci * H : ci * H + H],
                                lt[:, c % GL, ts(j, P)],
                                wkq_sb[:, j, 0:H],
                                start=(j == 0),
                                stop=(j == NJ - 1),
                            )
                    s_ts.append(s_t)

                def emit_exp(e):
                    p_t = ppp.tile([P, GE * H], f16)
                    nc.scalar.activation(
                        p_t,
                        s_ts[e],
                        mybir.ActivationFunctionType.Exp,
                        scale=SCALE,
                    )
                    p_ts.append(p_t)

                # x8 loads; exps slot into the scalar (ACT) queue after its
                # EXP_AFTER[k]-th x8-phase DMA so the softmax weights are
                # ready while the x8 stream is still in flight.
                exps_done = 0
                act_dma_count[0] = 0
                for g in range(NG):
                    if W_POS == "mid" and g == NG - 2:
                        emit_weights()
                    t = xin.tile([P, GL, D], f8, name=f"x8g{g}")
                    qi = pick()
                    engs[qi].dma_start(t, x8v[g])
                    load[qi] += 790.0
                    x8t[g] = t
                    while exps_done < NE and exps_done < len(
                        [k for k in EXP_AFTER if k <= act_dma_count[0]]
                    ):
                        emit_exp(exps_done)
                        exps_done += 1
                for e in range(exps_done, NE):
                    emit_exp(e)
                if W_POS == "late":
                    emit_weights()

                # Z (replicated over partitions) then xa accumulation, in
                # chunk order so the PE paces with the x8 stream.
                for c in range(NCHUNK):
                    nc.tensor.matmul(
                        z_ps,
                        ones_mat,
                        p_ts[c // GE][:, (c % GE) * H : (c % GE) * H + H],
                        start=(c == 0),
                        stop=(c == NCHUNK - 1),
                    )
                for c in range(NCHUNK):
                    xt_ = x8t[c // GL]
                    pr = p_ts[c // GE][:, (c % GE) * H : (c % GE) * H + H]
                    for j in range(NJ):
                        nc.tensor.matmul(
                            xa_ps[j],
                            xt_[:, c % GL, ts(j, P)],
                            pr,
                            start=(c == 0),
                            stop=(c == NCHUNK - 1),
                        )

                # ---- epilogue ----
                # xa drains split DVE/ACT; 1/Z on DVE afterwards (hides
                # under the PE's vtall matmuls).
                zrec_sb = epi.tile([P, H], f32)
                nc.vector.reciprocal(zrec_sb, z_ps)
                xa_sb = epi.tile([P, NJ, H], f16)
                nc.vector.tensor_copy(xa_sb[:, 0, :], xa_ps[0])
                nc.scalar.copy(xa_sb[:, 1, :], xa_ps[1])
                nc.vector.tensor_copy(xa_sb[:, 2, :], xa_ps[2])
                nc.scalar.copy(xa_sb[:, 3, :], xa_ps[3])

            with tc.tile_pool(name="pe1", bufs=1, space="PSUM") as pe1:
                # V^T-ish accumulator: vtall[p, 2j+c] = sum_d wv[d, 128j+p']
                # xa_n[d, 2j+c]; only the (c == p//64) columns matter.
                vtall_ps = pe1.tile([P, 2 * NJ], f32, tag="vtall")
                for j in range(NJ):
                    for k in range(NJ):
                        nc.tensor.matmul(
                            vtall_ps[:, 2 * j : 2 * j + 2],
                            wv_sb[:, k, ts(j, P)],
                            xa_sb[:, k, 2 * j : 2 * j + 2],
                            start=(k == 0),
                            stop=(k == NJ - 1),
                        )
                vtn_sb = epi.tile([P, 2 * NJ], f16)
                nc.vector.tensor_mul(vtn_sb, vtall_ps, zrec_sb)

                # row^T [128, 4]: rowT[p, nb] = row[128 nb + p]; the two
                # partition halves select even/odd head columns of vtn.
                rowT_ps = pe1.tile([P, NJ], f32, tag="rowt")
                for nb in range(NJ):
                    mm = 0
                    for j in range(NJ):
                        for hb in range(2):
                            sl = slice(64 * hb, 64 * hb + 64)
                            nc.tensor.matmul(
                                rowT_ps[:, nb : nb + 1],
                                wo_sb[sl, j, ts(nb, P)],
                                vtn_sb[sl, 2 * j + hb : 2 * j + hb + 1],
                                start=(mm == 0),
                                stop=(mm == 7),
                            )
                            mm += 1
                rowT_sb = epi.tile([P, NJ], f16)
                nc.vector.tensor_add(rowT_sb, rowT_ps, wkq_sb[:, :, H])

                # bounce row^T -> DRAM row, then one stride-0 broadcast DMA
                # (same queue, so its DGE delay overlaps the bounce) writes
                # all 8192 identical output rows.
                rview = bass.AP(row_d[:].tensor, 0, [[1, P], [P, NJ]])
                nc.sync.dma_start(rview, rowT_sb[:])
                nc.sync.dma_start(out_d[:], row_d[:].broadcast_to((L, D)))

    if not nc.is_finalized():
        nc.finalize()
    return nc


def _get_nc():
    if "nc" not in _CACHE:
        _CACHE["nc"] = _build_bass()
    return _CACHE["nc"]


def _host_prep(inputs):
    poi = np.asarray(inputs["poi_data"], np.float32)
    wq1 = np.asarray(inputs["wq1"], np.float32)
    bq1 = np.asarray(inputs["bq1"], np.float32)
    wq2 = np.asarray(inputs["wq2"], np.float32)
    bq2 = np.asarray(inputs["bq2"], np.float32)
    wk = np.asarray(inputs["wk"], np.float32)

    q1 = (poi @ wq1 + bq1)[:, 0]  # [1683]
    q = q1 @ wq2 + bq2  # [512]
    qh = q.reshape(H, DH)
    wkq = np.stack(
        [wk[:, h * DH : (h + 1) * DH] @ qh[h] for h in range(H)], axis=1
    )  # [512, 8]
    return wkq


def _make_in_maps(inputs):
    import ml_dtypes

    f8 = ml_dtypes.float8_e4m3
    wdt = f8 if W_FP8 else np.float16
    x = np.asarray(inputs["x"], np.float32)
    wv = np.asarray(inputs["wv"], np.float32).astype(wdt)
    wo32 = np.asarray(inputs["wo"], np.float32)
    wo = wo32.astype(wdt)
    bv = np.asarray(inputs["bv"], np.float32).reshape(D)
    bo = np.asarray(inputs["bo"], np.float32).reshape(D)
    bo2 = (bv @ wo32 + bo).reshape(D).astype(np.float32)
    # wkq plus a 9th column: bo2t[p, j] = bo2[128 j + p], head-block packed
    wkq9 = np.zeros((D, H + 1), np.float16)
    wkq9[:, 0:H] = _host_prep(inputs).astype(np.float16)
    # column H holds bo2 so that wkq9.rearrange((j p) h)[p, j, H] = bo2[128j+p]
    wkq9[:, H] = bo2.reshape(NJ, P).T.reshape(-1, order="F")

    x8 = x.astype(f8)  # [B, L, D]
    # transposed stream: row (n*128 + p) of xt8[b] holds, for chunk n and
    # d-block layout (j, q): xt[n, p, j*128 + q] = x[b, n*128 + q, j*128 + p]
    xt = np.ascontiguousarray(
        x8.reshape(B, NCHUNK, P, NJ, P).transpose(0, 1, 4, 3, 2)
    ).reshape(B, L, D)

    return [
        {
            "x8": np.ascontiguousarray(x8[b]),
            "xt8": xt[b],
            "wkq": wkq9,
            "wv": wv,
            "wo": wo,
        }
        for b in range(N_CORES)
    ]


def kernel(**inputs) -> np.ndarray:
    from concourse.bass_utils import run_bass_kernel_spmd

    nc = _get_nc()
    in_maps = _make_in_maps(inputs)
    res = run_bass_kernel_spmd(nc, in_maps, list(range(N_CORES)))
    out = np.stack([res.results[b]["out"] for b in range(N_CORES)], axis=0)
    return out.astype(np.float32)
